# revision 1
# baseline (speedup 1.0000x reference)
"""Expert-parallel MoE feed-forward for Trainium2 (8 NeuronCores).

Strategy:
  - Host: gate + top-2 routing (0.02% of FLOPs), builds per-expert token
    index lists.  Expert e is owned by core e.
  - Device (same SPMD program on all 8 cores): indirect-DMA gather of the
    expert's tokens, FFN  y = relu(x@W1+b1)@W2+b2  in float32r (full PE
    rate, ~1e-4 rel precision), scale by combine weight, write compact
    [C, D] result.
  - Host: scatter-add compact results into the [B,S,D] output.

All matmuls run in float32r: 1 cycle/row on the PE for moving dim >= 256,
fp32 storage, no cast passes (DRAM tensors are declared float32r and DMA'd
straight into float32r SBUF tiles).

Layout notes (per pass over TP tokens):
  xT[p, k*TP + t] = x[tok(t), k*128+p]     (PE-transposed gather)
  h [p, j*TP + t] = relu(x @ W1 + b1)[tok(t), j*128+p]
  mm1: out[F-chunk, tok] = W1[k,j].T @ xT[k]        (accumulate over k)
  mm2: out[tok, D-half] = h[j, m].T @ W2[j-chunk]   (accumulate over j)
  mm2 emits output directly in [token, D] rows -> plain DMA store.
"""

import numpy as np

B, S, D, F, E = 2, 2048, 1024, 4096, 8
T = B * S                      # 4096 tokens
K_TOP = 2
C = 1152                       # per-expert token capacity (9 * 128)
P = 128
PASSES = [(0, 640), (640, 512)]  # (token offset, pass size); sum == C

_CACHE = {}


def _build_program(loop_n=1, reps=1, mmdt="f32r"):
    import concourse.bass as bass
    import concourse.mybir as mybir
    import concourse.tile as tile
    from concourse import bacc
    from contextlib import ExitStack

    f32 = mybir.dt.float32
    f32r = mybir.dt.float32r if mmdt == "f32r" else mybir.dt.bfloat16
    i32 = mybir.dt.int32

    nc = bacc.Bacc("TRN2", target_bir_lowering=False, debug=False)

    x_d = nc.dram_tensor("x", [T, D], f32r, kind="ExternalInput").ap()
    w1_d = nc.dram_tensor("W1", [D, F], f32r, kind="ExternalInput").ap()
    w2_d = nc.dram_tensor("W2", [F, D], f32r, kind="ExternalInput").ap()
    idx_d = nc.dram_tensor("idx", [P, C // P], i32, kind="ExternalInput").ap()
    wc_d = nc.dram_tensor("wc", [P, C // P], f32, kind="ExternalInput").ap()
    b1_d = nc.dram_tensor("b1t", [P, F // P], f32, kind="ExternalInput").ap()
    # b2 replicated across partitions for the free-axis bias add
    b2_d = nc.dram_tensor("b2r", [P, D], f32, kind="ExternalInput").ap()
    idr_d = nc.dram_tensor("identr", [P, P], f32r, kind="ExternalInput").ap()
    y_d = nc.dram_tensor("yout", [C, D], f32, kind="ExternalOutput").ap()

    KD = D // P    # 8  k-tiles (mm1 contraction)
    NJ = F // P    # 32 f-tiles
    DH = D // 2    # 512, mm2 moving width

    with tile.TileContext(nc) as tc, ExitStack() as ctx:
        sb = ctx.enter_context(tc.tile_pool(name="sb", bufs=1))
        ps = ctx.enter_context(tc.tile_pool(name="ps", bufs=1, space="PSUM"))

        idx_t = sb.tile([P, C // P], i32, tag="idx")
        wc_t = sb.tile([P, C // P], f32, tag="wc")
        b1_t = sb.tile([P, F // P], f32, tag="b1")
        b2_t = sb.tile([P, D], f32, tag="b2")
        idr_t = sb.tile([P, P], f32r, tag="idr")
        nc.sync.dma_start(idx_t[:], idx_d[:])
        nc.sync.dma_start(wc_t[:], wc_d[:])
        nc.sync.dma_start(b1_t[:], b1_d[:])
        nc.sync.dma_start(b2_t[:], b2_d[:])
        nc.sync.dma_start(idr_t[:], idr_d[:])

        loop_cm = tc.For_i(0, loop_n, 1) if loop_n > 1 else None
        if loop_cm is not None:
            loop_cm.__enter__()

        xTs_all = []
        for rep in range(reps):
            # --- gather + transpose for BOTH passes up front (overlaps the
            # pass-A matmuls with pass-B's gather/transpose pipeline) ---
            xTs = {}
            xTs_all.append(xTs)
            for t0, TP in PASSES:
                NT = TP // P
                g0 = t0 // P
                xT = sb.tile([P, KD * TP], f32r, tag=f"xT{t0}", bufs=1,
                             name=f"xT_{rep}_{t0}")
                xTs[t0] = xT
                for g in range(NT):
                    xg = sb.tile([P, D], f32r, tag="xg", bufs=3,
                                 name=f"xg_{t0}_{g}")
                    nc.gpsimd.indirect_dma_start(
                        out=xg[:], out_offset=None,
                        in_=x_d[:],
                        in_offset=bass.IndirectOffsetOnAxis(
                            ap=idx_t[:, g0 + g:g0 + g + 1], axis=0),
                    )
                    for k in range(KD):
                        tp = ps.tile([P, P], f32r, tag="mm", bufs=8,
                                     name=f"tpx_{t0}_{g}_{k}")
                        nc.tensor.transpose(
                            out=tp[:], in_=xg[:, k * P:(k + 1) * P],
                            identity=idr_t[:])
                        nc.vector.tensor_copy(
                            xT[:, k * TP + g * P: k * TP + (g + 1) * P], tp[:])

        for rep, (t0, TP) in [(r, p) for r in range(reps) for p in PASSES]:
            NT = TP // P          # gather tiles in this pass
            NS = TP // 2          # mm1 moving size (320 / 256)
            g0 = t0 // P
            xT = xTs_all[rep][t0]

            # --- mm1 + relu ---
            h = sb.tile([P, NJ * TP], f32r, tag="h", bufs=1, name=f"h_{t0}")
            for j4 in range(NJ // 4):
                w1ts = []
                for k in range(KD):
                    w1t = sb.tile([P, 4 * P], f32r, tag="w1", bufs=10,
                                  name=f"w1_{t0}_{j4}_{k}")
                    nc.sync.dma_start(
                        w1t[:],
                        w1_d[k * P:(k + 1) * P, j4 * 4 * P:(j4 + 1) * 4 * P])
                    w1ts.append(w1t)
                for j2 in range(2):
                    acc = [[ps.tile([P, NS], f32, tag="mm", bufs=8,
                                    name=f"p1_{t0}_{j4}_{j2}_{jj}_{n}")
                            for n in range(2)] for jj in range(2)]
                    for k in range(KD):
                        for jj in range(2):
                            for n in range(2):
                                nc.tensor.matmul(
                                    acc[jj][n][:],
                                    lhsT=w1ts[k][:, (j2 * 2 + jj) * P:
                                                  (j2 * 2 + jj + 1) * P],
                                    rhs=xT[:, k * TP + n * NS:
                                           k * TP + (n + 1) * NS],
                                    start=(k == 0), stop=(k == KD - 1))
                    for jj in range(2):
                        j = j4 * 4 + j2 * 2 + jj
                        for n in range(2):
                            nc.scalar.activation(
                                h[:, j * TP + n * NS: j * TP + (n + 1) * NS],
                                acc[jj][n][:],
                                mybir.ActivationFunctionType.Relu,
                                bias=b1_t[:, j:j + 1])

            # --- mm2: out[tok, D] = sum_j h[j, m].T @ W2[j-chunk, :] ---
            m_groups = [(0, 3), (3, 2)] if NT == 5 else [(0, 4)]
            for m0, mcnt in m_groups:
                accs = {}
                for m in range(mcnt):
                    for dn in range(2):
                        accs[(m, dn)] = ps.tile(
                            [P, DH], f32, tag="mm", bufs=8,
                            name=f"p2_{t0}_{m0}_{m}_{dn}")
                for j in range(NJ):
                    w2c = sb.tile([P, D], f32r, tag="w2", bufs=4,
                                  name=f"w2_{t0}_{m0}_{j}")
                    nc.sync.dma_start(w2c[:], w2_d[j * P:(j + 1) * P, :])
                    for m in range(mcnt):
                        for dn in range(2):
                            nc.tensor.matmul(
                                accs[(m, dn)][:],
                                lhsT=h[:, j * TP + (m0 + m) * P:
                                       j * TP + (m0 + m + 1) * P],
                                rhs=w2c[:, dn * DH:(dn + 1) * DH],
                                start=(j == 0), stop=(j == NJ - 1))
                for m in range(mcnt):
                    g = m0 + m
                    for dn in range(2):
                        ot = sb.tile([P, DH], f32, tag="ot", bufs=4,
                                     name=f"ot_{t0}_{g}_{dn}")
                        nc.vector.tensor_tensor(
                            out=ot[:], in0=accs[(m, dn)][:],
                            in1=b2_t[:, dn * DH:(dn + 1) * DH],
                            op=mybir.AluOpType.add)
                        nc.vector.tensor_scalar_mul(
                            ot[:], ot[:], wc_t[:, g0 + g:g0 + g + 1])
                        nc.sync.dma_start(
                            y_d[t0 + g * P: t0 + (g + 1) * P,
                                dn * DH:(dn + 1) * DH], ot[:])

        if loop_cm is not None:
            loop_cm.__exit__(None, None, None)

    nc.compile()
    return nc


def _route(x2, Wg, bg):
    """Host-side top-2 routing in float64 (stable ordering)."""
    gate = x2.astype(np.float64) @ np.asarray(Wg, np.float64) + np.asarray(bg, np.float64)
    part = np.argpartition(-gate, K_TOP - 1, axis=1)[:, :K_TOP]      # [T, 2]
    rows = np.arange(T)[:, None]
    sc = gate[rows, part]                                            # [T, 2]
    sc = sc - sc.max(axis=1, keepdims=True)
    e_sc = np.exp(sc)
    probs = e_sc / e_sc.sum(axis=1, keepdims=True)                   # [T, 2]
    idx_e, w_e, n_e = [], [], []
    for e in range(E):
        mask = part == e                                             # [T, 2]
        tok = np.nonzero(mask.any(axis=1))[0]
        pr = probs[mask]                                             # aligned with tok
        n = len(tok)
        pad = C - n
        if pad < 0:
            return None                                              # capacity overflow
        idx_e.append(np.concatenate([tok, np.zeros(pad, np.int64)]).astype(np.int32))
        w_e.append(np.concatenate([pr, np.zeros(pad)]).astype(np.float32))
        n_e.append(n)
    return idx_e, w_e, n_e


def kernel(x, W1, b1, W2, b2, Wg, bg, num_experts_per_token):
    from concourse.bass_utils import run_bass_kernel_spmd

    x2 = np.ascontiguousarray(np.asarray(x, np.float32).reshape(T, D))
    W1 = np.asarray(W1, np.float32)
    b1 = np.asarray(b1, np.float32)
    W2 = np.asarray(W2, np.float32)
    b2 = np.asarray(b2, np.float32)

    routing = _route(x2, Wg, bg)
    if routing is None or int(num_experts_per_token) != K_TOP:
        # capacity overflow or unexpected top-k: correct slow path
        gate = x2.astype(np.float64) @ np.asarray(Wg, np.float64) + np.asarray(bg, np.float64)
        k = int(num_experts_per_token)
        part = np.argsort(-gate, axis=1)[:, :k]
        sc = gate[np.arange(T)[:, None], part]
        sc = sc - sc.max(axis=1, keepdims=True)
        pr = np.exp(sc); pr /= pr.sum(axis=1, keepdims=True)
        out = np.zeros((T, D), np.float32)
        for e in range(E):
            mask = part == e
            tok = np.nonzero(mask.any(axis=1))[0]
            w = pr[mask].astype(np.float32)
            hcur = np.maximum(x2[tok] @ W1[e] + b1[e], 0.0)
            out[tok] += w[:, None] * (hcur @ W2[e] + b2[e])
        return out.reshape(B, S, D)

    idx_e, w_e, n_e = routing

    if "nc" not in _CACHE:
        _CACHE["nc"] = _build_program()
    nc = _CACHE["nc"]

    ident = np.eye(P, dtype=np.float32)
    in_maps = []
    for e in range(E):
        in_maps.append({
            "x": x2,
            "W1": W1[e],
            "W2": W2[e],
            "idx": np.ascontiguousarray(idx_e[e].reshape(C // P, P).T),
            "wc": np.ascontiguousarray(w_e[e].reshape(C // P, P).T),
            "b1t": np.ascontiguousarray(b1[e].reshape(F // P, P).T),
            "b2r": np.ascontiguousarray(np.broadcast_to(b2[e], (P, D))),
            "identr": ident,
        })

    res = run_bass_kernel_spmd(nc, in_maps, list(range(E)))

    out = np.zeros((T, D), np.float32)
    for e in range(E):
        n = n_e[e]
        out[idx_e[e][:n]] += res.results[e]["yout"][:n]
    return out.reshape(B, S, D)



# revision 3
# speedup vs baseline: 1.0982x; 1.0982x over previous
"""Expert-parallel MoE feed-forward for Trainium2 (8 NeuronCores).

Strategy:
  - Host: gate + top-2 routing (0.02% of FLOPs), builds per-expert token
    index lists.  Expert e is owned by core e.
  - Device (same SPMD program on all 8 cores): indirect-DMA gather of the
    expert's tokens, FFN  y = relu(x@W1+b1)@W2+b2  in bf16 (full PE rate,
    fp32 PSUM accumulation), scale by combine weight, write y^T [D, C].
  - Host: scatter-add compact results into the [B,S,D] output.

Per-core pipeline (single pass over TUSE <= C token columns):
  W1 resident in SBUF (8 x 1MB DMAs, 8KB lines).
  gather tile g: xg[t, d] = x[tok(g*128+t), d]            (indirect DMA)
  PE-transpose:  xT_c[p, k*384 + g*128 + t] = xg[t, k*128+p]
  mm1: acc[f, tok] = sum_k W1[k-chunk, j-tile].T @ xT[k]  (fp32 PSUM)
       h[f, tok] = relu(acc + b1)                         (bf16, ACT)
  mm2: acc2[d, tok] = sum_j W2[j, d-tile].T @ h[j, tok]   (W2 stationary,
       streamed once via host-prepacked [P, KD*NJ*P] layout)
  y^T[d, tok] = (acc2 + b2) * wc[tok]                     (DVE), DMA out.
"""

import numpy as np

B, S, D, F, E = 2, 2048, 1024, 4096, 8
T = B * S                      # 4096 tokens
K_TOP = 2
P = 128
C = 1152                       # per-expert token capacity (9 * 128)
KD = D // P                    # 8 contraction tiles for mm1
NJ = F // P                    # 32 f-tiles
GT = C // P                    # 9 gather tiles
GPC = 3                        # gather tiles per chunk
CW = GPC * P                   # chunk width (384 token columns)

_CACHE = {}


def _chunks(tuse):
    out = []
    for t0 in range(0, tuse, CW):
        out.append((t0, min(CW, tuse - t0)))
    return out


def _build_program(tuse, loop_n=1):
    import concourse.bass as bass
    import concourse.mybir as mybir
    import concourse.tile as tile
    from concourse import bacc
    from contextlib import ExitStack

    f32 = mybir.dt.float32
    bf16 = mybir.dt.bfloat16
    i32 = mybir.dt.int32

    nc = bacc.Bacc("TRN2", target_bir_lowering=False, debug=False)

    x_d = nc.dram_tensor("x", [T, D], bf16, kind="ExternalInput").ap()
    w1_d = nc.dram_tensor("W1", [D, F], bf16, kind="ExternalInput").ap()
    # W2 prepacked on host: w2p[p, (d*NJ + j)*P + c] = W2[j*P + p, d*P + c]
    w2_d = nc.dram_tensor("W2p", [P, KD * NJ * P], bf16,
                          kind="ExternalInput").ap()
    idx_d = nc.dram_tensor("idx", [P, GT], i32, kind="ExternalInput").ap()
    # combine weights replicated across partitions: wcr[p, t] = wc[t]
    wcr_d = nc.dram_tensor("wcr", [P, C], f32, kind="ExternalInput").ap()
    b1_d = nc.dram_tensor("b1t", [P, NJ], f32, kind="ExternalInput").ap()
    b2_d = nc.dram_tensor("b2c", [P, KD], f32, kind="ExternalInput").ap()
    idb_d = nc.dram_tensor("identb", [P, P], bf16, kind="ExternalInput").ap()
    y_d = nc.dram_tensor("yout", [D, C], f32, kind="ExternalOutput").ap()

    chunks = _chunks(tuse)
    n_gt = (tuse + P - 1) // P

    with tile.TileContext(nc) as tc, ExitStack() as ctx:
        sb = ctx.enter_context(tc.tile_pool(name="sb", bufs=1))
        ps = ctx.enter_context(tc.tile_pool(name="ps", bufs=1, space="PSUM"))

        idx_t = sb.tile([P, GT], i32, tag="idx")
        wcr_t = sb.tile([P, C], f32, tag="wcr")
        b1_t = sb.tile([P, NJ], f32, tag="b1")
        b2_t = sb.tile([P, KD], f32, tag="b2")
        idb_t = sb.tile([P, P], bf16, tag="idb")
        nc.sync.dma_start(idx_t[:], idx_d[:])
        nc.sync.dma_start(wcr_t[:], wcr_d[:])
        nc.sync.dma_start(b1_t[:], b1_d[:])
        nc.sync.dma_start(b2_t[:], b2_d[:])
        nc.sync.dma_start(idb_t[:], idb_d[:])

        loop_cm = tc.For_i(0, loop_n, 1) if loop_n > 1 else None
        if loop_cm is not None:
            loop_cm.__enter__()

        # --- W1 resident: 8 DMAs of [128, F] (8KB per-partition lines) ---
        w1sb = sb.tile([P, KD * F], bf16, tag="w1", name="w1sb")
        for k in range(KD):
            nc.sync.dma_start(w1sb[:, k * F:(k + 1) * F],
                              w1_d[k * P:(k + 1) * P, :])

        # --- gather + PE transpose, per chunk ---
        xTs = []
        for ci, (t0, ns) in enumerate(chunks):
            xT = sb.tile([P, KD * CW], bf16, tag="xT", bufs=3,
                         name=f"xT_{ci}")
            xTs.append(xT)
            for g in range(GPC * ci, min(GPC * (ci + 1), n_gt)):
                gl = g - GPC * ci
                xg = sb.tile([P, D], bf16, tag="xg", bufs=3,
                             name=f"xg_{g}")
                nc.gpsimd.indirect_dma_start(
                    out=xg[:], out_offset=None,
                    in_=x_d[:],
                    in_offset=bass.IndirectOffsetOnAxis(
                        ap=idx_t[:, g:g + 1], axis=0),
                )
                for k in range(KD):
                    tp = ps.tile([P, P], bf16, tag="tp", bufs=3,
                                 name=f"tp_{g}_{k}")
                    nc.tensor.transpose(
                        out=tp[:], in_=xg[:, k * P:(k + 1) * P],
                        identity=idb_t[:])
                    nc.vector.tensor_copy(
                        xT[:, k * CW + gl * P: k * CW + (gl + 1) * P], tp[:])

        # --- mm1 + relu -> h ---
        h = sb.tile([P, NJ * C], bf16, tag="h", name="h")
        for ci, (t0, ns) in enumerate(chunks):
            xT = xTs[ci]
            for j in range(NJ):
                acc = ps.tile([P, ns], f32, tag="mm1", bufs=2,
                              name=f"p1_{ci}_{j}")
                for k in range(KD):
                    nc.tensor.matmul(
                        acc[:],
                        lhsT=w1sb[:, k * F + j * P: k * F + (j + 1) * P],
                        rhs=xT[:, k * CW: k * CW + ns],
                        start=(k == 0), stop=(k == KD - 1))
                nc.scalar.activation(
                    h[:, j * C + t0: j * C + t0 + ns], acc[:],
                    mybir.ActivationFunctionType.Relu,
                    bias=b1_t[:, j:j + 1])

        # --- mm2: y^T[d, tok] = sum_j W2[j, d].T @ h[j, tok] ---
        for d in range(KD):
            w2t = sb.tile([P, NJ * P], bf16, tag="w2", bufs=2,
                          name=f"w2_{d}")
            nc.sync.dma_start(w2t[:], w2_d[:, d * NJ * P:(d + 1) * NJ * P])
            for ci, (t0, ns) in enumerate(chunks):
                acc2 = ps.tile([P, ns], f32, tag="mm2", bufs=3,
                               name=f"p2_{d}_{ci}")
                for j in range(NJ):
                    nc.tensor.matmul(
                        acc2[:],
                        lhsT=w2t[:, j * P:(j + 1) * P],
                        rhs=h[:, j * C + t0: j * C + t0 + ns],
                        start=(j == 0), stop=(j == NJ - 1))
                ot = sb.tile([P, ns], f32, tag="ot", bufs=4,
                             name=f"ot_{d}_{ci}")
                nc.vector.tensor_scalar_add(ot[:], acc2[:], b2_t[:, d:d + 1])
                nc.vector.tensor_tensor(
                    out=ot[:], in0=ot[:], in1=wcr_t[:, t0:t0 + ns],
                    op=mybir.AluOpType.mult)
                nc.sync.dma_start(
                    y_d[d * P:(d + 1) * P, t0:t0 + ns], ot[:])

        if loop_cm is not None:
            loop_cm.__exit__(None, None, None)

    nc.compile()
    return nc


def _route(x2, Wg, bg):
    """Host-side top-2 routing in float64 (stable ordering)."""
    gate = x2.astype(np.float64) @ np.asarray(Wg, np.float64) + np.asarray(bg, np.float64)
    part = np.argpartition(-gate, K_TOP - 1, axis=1)[:, :K_TOP]      # [T, 2]
    rows = np.arange(T)[:, None]
    sc = gate[rows, part]                                            # [T, 2]
    sc = sc - sc.max(axis=1, keepdims=True)
    e_sc = np.exp(sc)
    probs = e_sc / e_sc.sum(axis=1, keepdims=True)                   # [T, 2]
    idx_e, w_e, n_e = [], [], []
    for e in range(E):
        mask = part == e                                             # [T, 2]
        tok = np.nonzero(mask.any(axis=1))[0]
        pr = probs[mask]                                             # aligned with tok
        n = len(tok)
        pad = C - n
        if pad < 0:
            return None                                              # capacity overflow
        idx_e.append(np.concatenate([tok, np.zeros(pad, np.int64)]).astype(np.int32))
        w_e.append(np.concatenate([pr, np.zeros(pad)]).astype(np.float32))
        n_e.append(n)
    return idx_e, w_e, n_e


def _prepack_w2(W2e_bf16):
    """[F, D] -> [P, KD*NJ*P]: w2p[p, (d*NJ+j)*P + c] = W2[j*P+p, d*P+c]."""
    w = W2e_bf16.reshape(NJ, P, KD, P)          # (j, p, d, c)
    w = w.transpose(1, 2, 0, 3)                 # (p, d, j, c)
    return np.ascontiguousarray(w.reshape(P, KD * NJ * P))


def kernel(x, W1, b1, W2, b2, Wg, bg, num_experts_per_token):
    import ml_dtypes
    from concourse.bass_utils import run_bass_kernel_spmd

    bf16 = ml_dtypes.bfloat16
    x2 = np.ascontiguousarray(np.asarray(x, np.float32).reshape(T, D))
    W1 = np.asarray(W1, np.float32)
    b1 = np.asarray(b1, np.float32)
    W2 = np.asarray(W2, np.float32)
    b2 = np.asarray(b2, np.float32)

    routing = _route(x2, Wg, bg)
    if routing is None or int(num_experts_per_token) != K_TOP:
        # capacity overflow or unexpected top-k: correct slow path
        gate = x2.astype(np.float64) @ np.asarray(Wg, np.float64) + np.asarray(bg, np.float64)
        k = int(num_experts_per_token)
        part = np.argsort(-gate, axis=1)[:, :k]
        sc = gate[np.arange(T)[:, None], part]
        sc = sc - sc.max(axis=1, keepdims=True)
        pr = np.exp(sc); pr /= pr.sum(axis=1, keepdims=True)
        out = np.zeros((T, D), np.float32)
        for e in range(E):
            mask = part == e
            tok = np.nonzero(mask.any(axis=1))[0]
            w = pr[mask].astype(np.float32)
            hcur = np.maximum(x2[tok] @ W1[e] + b1[e], 0.0)
            out[tok] += w[:, None] * (hcur @ W2[e] + b2[e])
        return out.reshape(B, S, D)

    idx_e, w_e, n_e = routing
    tuse = min(C, ((max(n_e) + 3) // 4) * 4)

    key = ("nc", tuse)
    if key not in _CACHE:
        _CACHE[key] = _build_program(tuse)
    nc = _CACHE[key]

    x_bf = x2.astype(bf16)
    ident = np.eye(P, dtype=bf16)
    in_maps = []
    for e in range(E):
        in_maps.append({
            "x": x_bf,
            "W1": W1[e].astype(bf16),
            "W2p": _prepack_w2(W2[e].astype(bf16)),
            "idx": np.ascontiguousarray(idx_e[e].reshape(GT, P).T),
            "wcr": np.ascontiguousarray(
                np.broadcast_to(w_e[e], (P, C))).astype(np.float32),
            "b1t": np.ascontiguousarray(b1[e].reshape(NJ, P).T),
            "b2c": np.ascontiguousarray(b2[e].reshape(KD, P).T),
            "identb": ident,
        })

    res = run_bass_kernel_spmd(nc, in_maps, list(range(E)))

    out = np.zeros((T, D), np.float32)
    for e in range(E):
        n = n_e[e]
        out[idx_e[e][:n]] += res.results[e]["yout"][:, :n].T
    return out.reshape(B, S, D)


# revision 7
# speedup vs baseline: 1.1516x; 1.0487x over previous
"""Expert-parallel MoE feed-forward for Trainium2 (8 NeuronCores).

Strategy:
  - Host: gate + top-2 routing (0.02% of FLOPs), builds per-expert token
    index lists.  Expert e is owned by core e.
  - Device (same SPMD program on all 8 cores): indirect-DMA gather of the
    expert's tokens, FFN  y = relu(x@W1+b1)@W2+b2  in bf16 (full PE rate,
    fp32 PSUM accumulation), scale by combine weight, write y^T [D, C].
  - Host: scatter-add compact results into the [B,S,D] output.

Per-core pipeline (single pass over TUSE <= C token columns):
  W1 resident in SBUF (8 x 1MB DMAs, 8KB lines).
  gather tile g: xg[t, d] = x[tok(g*128+t), d]            (indirect DMA)
  PE-transpose:  xT_c[p, k*384 + g*128 + t] = xg[t, k*128+p]
  mm1: acc[f, tok] = sum_k W1[k-chunk, j-tile].T @ xT[k]  (fp32 PSUM)
       h[f, tok] = relu(acc + b1)                         (bf16, ACT)
  mm2: acc2[d, tok] = sum_j W2[j, d-tile].T @ h[j, tok]   (W2 stationary,
       streamed once via host-prepacked [P, KD*NJ*P] layout)
  y^T[d, tok] = (acc2 + b2) * wc[tok]                     (DVE), DMA out.
"""

import numpy as np

B, S, D, F, E = 2, 2048, 1024, 4096, 8
T = B * S                      # 4096 tokens
K_TOP = 2
P = 128
C = 1152                       # per-expert token capacity (9 * 128)
KD = D // P                    # 8 contraction tiles for mm1
NJ = F // P                    # 32 f-tiles
GT = C // P                    # 9 gather tiles
GPC = 3                        # gather tiles per chunk
CW = GPC * P                   # chunk width (384 token columns)

_CACHE = {}


def _chunks(tuse):
    out = []
    for t0 in range(0, tuse, CW):
        out.append((t0, min(CW, tuse - t0)))
    return out


def _build_program(tuse, loop_n=1):
    import concourse.bass as bass
    import concourse.mybir as mybir
    import concourse.tile as tile
    from concourse import bacc
    from contextlib import ExitStack

    f32 = mybir.dt.float32
    bf16 = mybir.dt.bfloat16
    i32 = mybir.dt.int32

    nc = bacc.Bacc("TRN2", target_bir_lowering=False, debug=False)

    x_d = nc.dram_tensor("x", [T, D], bf16, kind="ExternalInput").ap()
    # W1 prepacked on host: w1p[p, (j*KD + k)*P + c] = W1[k*P + p, j*P + c]
    w1_d = nc.dram_tensor("W1p", [P, NJ * KD * P], bf16,
                          kind="ExternalInput").ap()
    # W2 prepacked on host: w2p[p, (d*NJ + j)*P + c] = W2[j*P + p, d*P + c]
    w2_d = nc.dram_tensor("W2p", [P, KD * NJ * P], bf16,
                          kind="ExternalInput").ap()
    idx_d = nc.dram_tensor("idx", [P, GT], i32, kind="ExternalInput").ap()
    # combine weights replicated across partitions: wcr[p, t] = wc[t]
    wcr_d = nc.dram_tensor("wcr", [P, C], f32, kind="ExternalInput").ap()
    b1_d = nc.dram_tensor("b1t", [P, NJ], f32, kind="ExternalInput").ap()
    b2_d = nc.dram_tensor("b2c", [P, KD], f32, kind="ExternalInput").ap()
    idb_d = nc.dram_tensor("identb", [P, P], bf16, kind="ExternalInput").ap()
    y_d = nc.dram_tensor("yout", [D, C], f32, kind="ExternalOutput").ap()

    chunks = _chunks(tuse)
    n_gt = (tuse + P - 1) // P

    with tile.TileContext(nc) as tc, ExitStack() as ctx:
        sb = ctx.enter_context(tc.tile_pool(name="sb", bufs=1))
        ps = ctx.enter_context(tc.tile_pool(name="ps", bufs=1, space="PSUM"))

        idx_t = sb.tile([P, GT], i32, tag="idx")
        wcr_t = sb.tile([P, C], f32, tag="wcr")
        b1_t = sb.tile([P, NJ], f32, tag="b1")
        b2_t = sb.tile([P, KD], f32, tag="b2")
        idb_t = sb.tile([P, P], bf16, tag="idb")
        nc.sync.dma_start(idx_t[:], idx_d[:])
        nc.sync.dma_start(wcr_t[:], wcr_d[:])
        nc.sync.dma_start(b1_t[:], b1_d[:])
        nc.sync.dma_start(b2_t[:], b2_d[:])
        nc.sync.dma_start(idb_t[:], idb_d[:])

        loop_cm = tc.For_i(0, loop_n, 1) if loop_n > 1 else None
        if loop_cm is not None:
            loop_cm.__enter__()

        # --- W1: 32 granular j-tiles (2KB per-partition lines each) ---
        w1t = []
        for j in range(NJ):
            t = sb.tile([P, KD * P], bf16, tag="w1", bufs=NJ + 2,
                        name=f"w1_{j}")
            nc.sync.dma_start(t[:], w1_d[:, j * KD * P:(j + 1) * KD * P])
            w1t.append(t)

        # --- per chunk: gather + PE transpose, then mm1 + relu -> h ---
        # (interleaved so PE never has a long matmul-free window)
        h = sb.tile([P, NJ * C], bf16, tag="h", name="h")
        for ci, (t0, ns) in enumerate(chunks):
            xT = sb.tile([P, KD * CW], bf16, tag="xT", bufs=3,
                         name=f"xT_{ci}")
            for g in range(GPC * ci, min(GPC * (ci + 1), n_gt)):
                gl = g - GPC * ci
                xg = sb.tile([P, D], bf16, tag="xg", bufs=3,
                             name=f"xg_{g}")
                nc.gpsimd.indirect_dma_start(
                    out=xg[:], out_offset=None,
                    in_=x_d[:],
                    in_offset=bass.IndirectOffsetOnAxis(
                        ap=idx_t[:, g:g + 1], axis=0),
                )
                for k in range(KD):
                    tp = ps.tile([P, P], bf16, tag="tp", bufs=2,
                                 name=f"tp_{g}_{k}")
                    nc.tensor.transpose(
                        out=tp[:], in_=xg[:, k * P:(k + 1) * P],
                        identity=idb_t[:])
                    nc.vector.tensor_copy(
                        xT[:, k * CW + gl * P: k * CW + (gl + 1) * P], tp[:])
            for j in range(NJ):
                acc = ps.tile([P, ns], f32, tag="mm1", bufs=3,
                              name=f"p1_{ci}_{j}")
                for k in range(KD):
                    nc.tensor.matmul(
                        acc[:],
                        lhsT=w1t[j][:, k * P:(k + 1) * P],
                        rhs=xT[:, k * CW: k * CW + ns],
                        start=(k == 0), stop=(k == KD - 1))
                nc.scalar.activation(
                    h[:, j * C + t0: j * C + t0 + ns], acc[:],
                    mybir.ActivationFunctionType.Relu,
                    bias=b1_t[:, j:j + 1])

        # --- mm2: y^T[d, tok] = sum_j W2[j, d].T @ h[j, tok] ---
        for d in range(KD):
            w2t = sb.tile([P, NJ * P], bf16, tag="w2", bufs=2,
                          name=f"w2_{d}")
            nc.sync.dma_start(w2t[:], w2_d[:, d * NJ * P:(d + 1) * NJ * P])
            for ci, (t0, ns) in enumerate(chunks):
                acc2 = ps.tile([P, ns], f32, tag="mm2", bufs=3,
                               name=f"p2_{d}_{ci}")
                for j in range(NJ):
                    nc.tensor.matmul(
                        acc2[:],
                        lhsT=w2t[:, j * P:(j + 1) * P],
                        rhs=h[:, j * C + t0: j * C + t0 + ns],
                        start=(j == 0), stop=(j == NJ - 1))
                ot = sb.tile([P, ns], f32, tag="ot", bufs=4,
                             name=f"ot_{d}_{ci}")
                nc.vector.tensor_scalar_add(ot[:], acc2[:], b2_t[:, d:d + 1])
                nc.vector.tensor_tensor(
                    out=ot[:], in0=ot[:], in1=wcr_t[:, t0:t0 + ns],
                    op=mybir.AluOpType.mult)
                nc.sync.dma_start(
                    y_d[d * P:(d + 1) * P, t0:t0 + ns], ot[:])

        if loop_cm is not None:
            loop_cm.__exit__(None, None, None)

    nc.compile()
    return nc


def _route(x2, Wg, bg):
    """Host-side top-2 routing in float64 (stable ordering)."""
    gate = x2.astype(np.float64) @ np.asarray(Wg, np.float64) + np.asarray(bg, np.float64)
    part = np.argpartition(-gate, K_TOP - 1, axis=1)[:, :K_TOP]      # [T, 2]
    rows = np.arange(T)[:, None]
    sc = gate[rows, part]                                            # [T, 2]
    sc = sc - sc.max(axis=1, keepdims=True)
    e_sc = np.exp(sc)
    probs = e_sc / e_sc.sum(axis=1, keepdims=True)                   # [T, 2]
    idx_e, w_e, n_e = [], [], []
    for e in range(E):
        mask = part == e                                             # [T, 2]
        tok = np.nonzero(mask.any(axis=1))[0]
        pr = probs[mask]                                             # aligned with tok
        n = len(tok)
        pad = C - n
        if pad < 0:
            return None                                              # capacity overflow
        idx_e.append(np.concatenate([tok, np.zeros(pad, np.int64)]).astype(np.int32))
        w_e.append(np.concatenate([pr, np.zeros(pad)]).astype(np.float32))
        n_e.append(n)
    return idx_e, w_e, n_e


def _prepack_w2(W2e_bf16):
    """[F, D] -> [P, KD*NJ*P]: w2p[p, (d*NJ+j)*P + c] = W2[j*P+p, d*P+c]."""
    w = W2e_bf16.reshape(NJ, P, KD, P)          # (j, p, d, c)
    w = w.transpose(1, 2, 0, 3)                 # (p, d, j, c)
    return np.ascontiguousarray(w.reshape(P, KD * NJ * P))


def _prepack_w1(W1e_bf16):
    """[D, F] -> [P, NJ*KD*P]: w1p[p, (j*KD+k)*P + c] = W1[k*P+p, j*P+c]."""
    w = W1e_bf16.reshape(KD, P, NJ, P)          # (k, p, j, c)
    w = w.transpose(1, 2, 0, 3)                 # (p, j, k, c)
    return np.ascontiguousarray(w.reshape(P, NJ * KD * P))


def kernel(x, W1, b1, W2, b2, Wg, bg, num_experts_per_token):
    import ml_dtypes
    from concourse.bass_utils import run_bass_kernel_spmd

    bf16 = ml_dtypes.bfloat16
    x2 = np.ascontiguousarray(np.asarray(x, np.float32).reshape(T, D))
    W1 = np.asarray(W1, np.float32)
    b1 = np.asarray(b1, np.float32)
    W2 = np.asarray(W2, np.float32)
    b2 = np.asarray(b2, np.float32)

    routing = _route(x2, Wg, bg)
    if routing is None or int(num_experts_per_token) != K_TOP:
        # capacity overflow or unexpected top-k: correct slow path
        gate = x2.astype(np.float64) @ np.asarray(Wg, np.float64) + np.asarray(bg, np.float64)
        k = int(num_experts_per_token)
        part = np.argsort(-gate, axis=1)[:, :k]
        sc = gate[np.arange(T)[:, None], part]
        sc = sc - sc.max(axis=1, keepdims=True)
        pr = np.exp(sc); pr /= pr.sum(axis=1, keepdims=True)
        out = np.zeros((T, D), np.float32)
        for e in range(E):
            mask = part == e
            tok = np.nonzero(mask.any(axis=1))[0]
            w = pr[mask].astype(np.float32)
            hcur = np.maximum(x2[tok] @ W1[e] + b1[e], 0.0)
            out[tok] += w[:, None] * (hcur @ W2[e] + b2[e])
        return out.reshape(B, S, D)

    idx_e, w_e, n_e = routing
    tuse = min(C, ((max(n_e) + 3) // 4) * 4)

    key = ("nc", tuse)
    if key not in _CACHE:
        _CACHE[key] = _build_program(tuse)
    nc = _CACHE[key]

    x_bf = x2.astype(bf16)
    ident = np.eye(P, dtype=bf16)
    in_maps = []
    for e in range(E):
        in_maps.append({
            "x": x_bf,
            "W1p": _prepack_w1(W1[e].astype(bf16)),
            "W2p": _prepack_w2(W2[e].astype(bf16)),
            "idx": np.ascontiguousarray(idx_e[e].reshape(GT, P).T),
            "wcr": np.ascontiguousarray(
                np.broadcast_to(w_e[e], (P, C))).astype(np.float32),
            "b1t": np.ascontiguousarray(b1[e].reshape(NJ, P).T),
            "b2c": np.ascontiguousarray(b2[e].reshape(KD, P).T),
            "identb": ident,
        })

    res = run_bass_kernel_spmd(nc, in_maps, list(range(E)))

    out = np.zeros((T, D), np.float32)
    for e in range(E):
        n = n_e[e]
        out[idx_e[e][:n]] += res.results[e]["yout"][:, :n].T
    return out.reshape(B, S, D)


# revision 10
# speedup vs baseline: 1.1629x; 1.0098x over previous
"""Expert-parallel MoE feed-forward for Trainium2 (8 NeuronCores).

Strategy:
  - Host: gate + top-2 routing (0.02% of FLOPs), builds per-expert token
    index lists.  Expert e is owned by core e.
  - Device (same SPMD program on all 8 cores): indirect-DMA gather of the
    expert's tokens, FFN  y = relu(x@W1+b1)@W2+b2  in bf16 (full PE rate,
    fp32 PSUM accumulation), scale by combine weight, write y^T [D, C].
  - Host: scatter-add compact results into the [B,S,D] output.

Per-core pipeline (single pass over TUSE <= C token columns):
  W1 resident in SBUF (8 x 1MB DMAs, 8KB lines).
  gather tile g: xg[t, d] = x[tok(g*128+t), d]            (indirect DMA)
  PE-transpose:  xT_c[p, k*384 + g*128 + t] = xg[t, k*128+p]
  mm1: acc[f, tok] = sum_k W1[k-chunk, j-tile].T @ xT[k]  (fp32 PSUM)
       h[f, tok] = relu(acc + b1)                         (bf16, ACT)
  mm2: acc2[d, tok] = sum_j W2[j, d-tile].T @ h[j, tok]   (W2 stationary,
       streamed once via host-prepacked [P, KD*NJ*P] layout)
  y^T[d, tok] = (acc2 + b2) * wc[tok]                     (DVE), DMA out.
"""

import numpy as np

B, S, D, F, E = 2, 2048, 1024, 4096, 8
T = B * S                      # 4096 tokens
K_TOP = 2
P = 128
C = 1152                       # per-expert token capacity (9 * 128)
KD = D // P                    # 8 contraction tiles for mm1
NJ = F // P                    # 32 f-tiles
GT = C // P                    # 9 gather tiles
GPC = 3                        # gather tiles per chunk
CW = GPC * P                   # chunk width (384 token columns)

_CACHE = {}


def _chunks(tuse):
    out = []
    for t0 in range(0, tuse, CW):
        out.append((t0, min(CW, tuse - t0)))
    return out


def _build_program(tuse, loop_n=1):
    import concourse.bass as bass
    import concourse.mybir as mybir
    import concourse.tile as tile
    from concourse import bacc
    from contextlib import ExitStack

    f32 = mybir.dt.float32
    bf16 = mybir.dt.bfloat16
    i32 = mybir.dt.int32

    nc = bacc.Bacc("TRN2", target_bir_lowering=False, debug=False)

    x_d = nc.dram_tensor("x", [T, D], bf16, kind="ExternalInput").ap()
    # W1 prepacked on host: w1p[p, (j*KD + k)*P + c] = W1[k*P + p, j*P + c]
    w1_d = nc.dram_tensor("W1p", [P, NJ * KD * P], bf16,
                          kind="ExternalInput").ap()
    # W2 prepacked on host: w2p[p, (d*NJ + j)*P + c] = W2[j*P + p, d*P + c]
    w2_d = nc.dram_tensor("W2p", [P, KD * NJ * P], bf16,
                          kind="ExternalInput").ap()
    idx_d = nc.dram_tensor("idx", [P, GT], i32, kind="ExternalInput").ap()
    # combine weights replicated across partitions: wcr[p, t] = wc[t]
    wcr_d = nc.dram_tensor("wcr", [P, C], f32, kind="ExternalInput").ap()
    b1_d = nc.dram_tensor("b1t", [P, NJ], f32, kind="ExternalInput").ap()
    b2_d = nc.dram_tensor("b2c", [P, KD], f32, kind="ExternalInput").ap()
    y_d = nc.dram_tensor("yout", [D, C], f32, kind="ExternalOutput").ap()

    chunks = _chunks(tuse)
    n_gt = (tuse + P - 1) // P

    with tile.TileContext(nc) as tc, ExitStack() as ctx:
        sb = ctx.enter_context(tc.tile_pool(name="sb", bufs=1))
        ps = ctx.enter_context(tc.tile_pool(name="ps", bufs=1, space="PSUM"))

        idx_t = sb.tile([P, GT], i32, tag="idx")
        wcr_t = sb.tile([P, C], f32, tag="wcr")
        b1_t = sb.tile([P, NJ], f32, tag="b1")
        b2_t = sb.tile([P, KD], f32, tag="b2")
        nc.sync.dma_start(idx_t[:], idx_d[:])
        nc.sync.dma_start(wcr_t[:], wcr_d[:])
        nc.sync.dma_start(b1_t[:], b1_d[:])
        nc.sync.dma_start(b2_t[:], b2_d[:])

        loop_cm = tc.For_i(0, loop_n, 1) if loop_n > 1 else None
        if loop_cm is not None:
            loop_cm.__enter__()

        # --- W1: 32 granular j-tiles (2KB per-partition lines each) ---
        w1t = []
        for j in range(NJ):
            t = sb.tile([P, KD * P], bf16, tag="w1", bufs=NJ + 2,
                        name=f"w1_{j}")
            nc.sync.dma_start(t[:], w1_d[:, j * KD * P:(j + 1) * KD * P])
            w1t.append(t)

        # --- per chunk: gather + PE transpose, then mm1 + relu -> h ---
        # (interleaved so PE never has a long matmul-free window)
        h = sb.tile([P, NJ * C], bf16, tag="h", name="h")
        for ci, (t0, ns) in enumerate(chunks):
            xT = sb.tile([P, KD * CW], bf16, tag="xT", bufs=3,
                         name=f"xT_{ci}")
            for g in range(GPC * ci, min(GPC * (ci + 1), n_gt)):
                gl = g - GPC * ci
                xg = sb.tile([P, D], bf16, tag="xg", bufs=3,
                             name=f"xg_{g}")
                nc.gpsimd.indirect_dma_start(
                    out=xg[:], out_offset=None,
                    in_=x_d[:],
                    in_offset=bass.IndirectOffsetOnAxis(
                        ap=idx_t[:, g:g + 1], axis=0),
                )
                # DMA-XBAR transpose straight into the k-strided xT layout
                nc.sync.dma_start_transpose(
                    xT[:].rearrange("p (k t) -> p k t", k=KD)
                         [:, :, gl * P:(gl + 1) * P],
                    xg[:])
            for j in range(NJ):
                acc = ps.tile([P, ns], f32, tag="mm1", bufs=4,
                              name=f"p1_{ci}_{j}")
                for k in range(KD):
                    nc.tensor.matmul(
                        acc[:],
                        lhsT=w1t[j][:, k * P:(k + 1) * P],
                        rhs=xT[:, k * CW: k * CW + ns],
                        start=(k == 0), stop=(k == KD - 1))
                nc.scalar.activation(
                    h[:, j * C + t0: j * C + t0 + ns], acc[:],
                    mybir.ActivationFunctionType.Relu,
                    bias=b1_t[:, j:j + 1])

        # --- mm2: y^T[d, tok] = sum_j W2[j, d].T @ h[j, tok] ---
        for d in range(KD):
            w2t = sb.tile([P, NJ * P], bf16, tag="w2", bufs=2,
                          name=f"w2_{d}")
            nc.sync.dma_start(w2t[:], w2_d[:, d * NJ * P:(d + 1) * NJ * P])
            for ci, (t0, ns) in enumerate(chunks):
                acc2 = ps.tile([P, ns], f32, tag="mm2", bufs=4,
                               name=f"p2_{d}_{ci}")
                for j in range(NJ):
                    nc.tensor.matmul(
                        acc2[:],
                        lhsT=w2t[:, j * P:(j + 1) * P],
                        rhs=h[:, j * C + t0: j * C + t0 + ns],
                        start=(j == 0), stop=(j == NJ - 1))
                ot = sb.tile([P, ns], f32, tag="ot", bufs=4,
                             name=f"ot_{d}_{ci}")
                nc.vector.tensor_scalar_add(ot[:], acc2[:], b2_t[:, d:d + 1])
                nc.vector.tensor_tensor(
                    out=ot[:], in0=ot[:], in1=wcr_t[:, t0:t0 + ns],
                    op=mybir.AluOpType.mult)
                nc.sync.dma_start(
                    y_d[d * P:(d + 1) * P, t0:t0 + ns], ot[:])

        if loop_cm is not None:
            loop_cm.__exit__(None, None, None)

    nc.compile()
    return nc


def _route(x2, Wg, bg):
    """Host-side top-2 routing in float64 (stable ordering)."""
    gate = x2.astype(np.float64) @ np.asarray(Wg, np.float64) + np.asarray(bg, np.float64)
    part = np.argpartition(-gate, K_TOP - 1, axis=1)[:, :K_TOP]      # [T, 2]
    rows = np.arange(T)[:, None]
    sc = gate[rows, part]                                            # [T, 2]
    sc = sc - sc.max(axis=1, keepdims=True)
    e_sc = np.exp(sc)
    probs = e_sc / e_sc.sum(axis=1, keepdims=True)                   # [T, 2]
    idx_e, w_e, n_e = [], [], []
    for e in range(E):
        mask = part == e                                             # [T, 2]
        tok = np.nonzero(mask.any(axis=1))[0]
        pr = probs[mask]                                             # aligned with tok
        n = len(tok)
        pad = C - n
        if pad < 0:
            return None                                              # capacity overflow
        idx_e.append(np.concatenate([tok, np.zeros(pad, np.int64)]).astype(np.int32))
        w_e.append(np.concatenate([pr, np.zeros(pad)]).astype(np.float32))
        n_e.append(n)
    return idx_e, w_e, n_e


def _prepack_w2(W2e_bf16):
    """[F, D] -> [P, KD*NJ*P]: w2p[p, (d*NJ+j)*P + c] = W2[j*P+p, d*P+c]."""
    w = W2e_bf16.reshape(NJ, P, KD, P)          # (j, p, d, c)
    w = w.transpose(1, 2, 0, 3)                 # (p, d, j, c)
    return np.ascontiguousarray(w.reshape(P, KD * NJ * P))


def _prepack_w1(W1e_bf16):
    """[D, F] -> [P, NJ*KD*P]: w1p[p, (j*KD+k)*P + c] = W1[k*P+p, j*P+c]."""
    w = W1e_bf16.reshape(KD, P, NJ, P)          # (k, p, j, c)
    w = w.transpose(1, 2, 0, 3)                 # (p, j, k, c)
    return np.ascontiguousarray(w.reshape(P, NJ * KD * P))


def kernel(x, W1, b1, W2, b2, Wg, bg, num_experts_per_token):
    import ml_dtypes
    from concourse.bass_utils import run_bass_kernel_spmd

    bf16 = ml_dtypes.bfloat16
    x2 = np.ascontiguousarray(np.asarray(x, np.float32).reshape(T, D))
    W1 = np.asarray(W1, np.float32)
    b1 = np.asarray(b1, np.float32)
    W2 = np.asarray(W2, np.float32)
    b2 = np.asarray(b2, np.float32)

    routing = _route(x2, Wg, bg)
    if routing is None or int(num_experts_per_token) != K_TOP:
        # capacity overflow or unexpected top-k: correct slow path
        gate = x2.astype(np.float64) @ np.asarray(Wg, np.float64) + np.asarray(bg, np.float64)
        k = int(num_experts_per_token)
        part = np.argsort(-gate, axis=1)[:, :k]
        sc = gate[np.arange(T)[:, None], part]
        sc = sc - sc.max(axis=1, keepdims=True)
        pr = np.exp(sc); pr /= pr.sum(axis=1, keepdims=True)
        out = np.zeros((T, D), np.float32)
        for e in range(E):
            mask = part == e
            tok = np.nonzero(mask.any(axis=1))[0]
            w = pr[mask].astype(np.float32)
            hcur = np.maximum(x2[tok] @ W1[e] + b1[e], 0.0)
            out[tok] += w[:, None] * (hcur @ W2[e] + b2[e])
        return out.reshape(B, S, D)

    idx_e, w_e, n_e = routing
    tuse = min(C, ((max(n_e) + 3) // 4) * 4)

    key = ("nc", tuse)
    if key not in _CACHE:
        _CACHE[key] = _build_program(tuse)
    nc = _CACHE[key]

    x_bf = x2.astype(bf16)
    in_maps = []
    for e in range(E):
        in_maps.append({
            "x": x_bf,
            "W1p": _prepack_w1(W1[e].astype(bf16)),
            "W2p": _prepack_w2(W2[e].astype(bf16)),
            "idx": np.ascontiguousarray(idx_e[e].reshape(GT, P).T),
            "wcr": np.ascontiguousarray(
                np.broadcast_to(w_e[e], (P, C))).astype(np.float32),
            "b1t": np.ascontiguousarray(b1[e].reshape(NJ, P).T),
            "b2c": np.ascontiguousarray(b2[e].reshape(KD, P).T),
        })

    res = run_bass_kernel_spmd(nc, in_maps, list(range(E)))

    out = np.zeros((T, D), np.float32)
    for e in range(E):
        n = n_e[e]
        out[idx_e[e][:n]] += res.results[e]["yout"][:, :n].T
    return out.reshape(B, S, D)
